# revision 49
# baseline (speedup 1.0000x reference)
"""Trainium2 Bass kernel for nn_BiEvidenceNet.

Model (B=1024, R=512, D=256):
    width  = clip(exp(log_width), 1e-3, 50)                  (R,D)
    t_low  = center - width/2 ; t_high = center + width/2    (R,D)
    kappa  = clip(exp(log_kappa), 0.5, 50)                   scalar
    low    = sigmoid(kappa*(t_low - x))   high = sigmoid(kappa*(x - t_high))
    evidence[b,r] = sum_d m*(el*(2*low-1) + eh*(2*high-1))   m=sig(mask), el/eh=tanh(e_*)
    z = sigmoid(6*(evidence - t));  y = z @ head_w.T + head_b

Key identity: 2*sigmoid(u)-1 = tanh(u/2).  When t_low / t_high are constant
across the rule axis (true at init; verified at runtime), the (B,R,D)
broadcast collapses to two matmuls over the feature dim:
    evidence^T = A^T_{d,r} @ tanh(k/2*(tau_lo - x))^T + B^T @ tanh(k/2*(x - tau_hi))^T
with A = sig(mask)*tanh(e_low), B = sig(mask)*tanh(e_high) folded on the host
(they are pure parameter transforms, O(R*D)).

On-core layout is fully transposed vs. the obvious one: the contraction (d)
lives on partitions for the tanh tiles and A/B, and evidence lands in PSUM
with RULES on partitions and batch on the free axis.  That makes t a
per-partition bias column of the z sigmoid, and the head a single stationary
column (head_w) matmul -> y arrives as [1, B2] in PSUM, one DMA descriptor.

The second tanh operand is derived on-device: x_hi = -x_lo + c with
c[d] = kappa/2*(tau_lo - tau_hi), so only ONE x tensor is DMA'd; everything
moves as bf16 (tolerance is 2e-2; measured end-to-end rel err ~3e-3).

Sharding: 4 batch shards x 2 rule shards over the 8 cores; rule-sharded
partial y vectors are summed on the host (with head_b) during the gather.

Perf structure (walrus encodes at most ONE sync wait per instruction, so the
whole graph is arranged as single-semaphore chains; measured ~15.0us vs the
25.2us baseline):
  - Input DMAs fire PRE-TileContext with manual completion semaphores, so
    their triggers and DGE generation overlap the fixed ~6.5us NEFF startup;
    3 parallel queues (scalar HWDGE: xlo first - that engine reaches its
    trigger earliest; sync HWDGE: ab_k0+consts; gpsimd SWDGE: ab_k1), sharing
    ~130-200GB/s of fabric, so total bytes (~390KB/core) sets the phase.
  - ACT: warm tanh pulls the 1.3us table load into the DMA shadow; tanh_lo
    and (when c is uniform) tanh_hi are each ONE merged activation; z
    sigmoids take t and BETA via per-partition bias columns.
  - PE: bf16 everywhere; right-sized dummy matmuls during the DMA shadow
    keep the p-state ramped without delaying the real matmuls; explicit
    post-scheduled sem-waits (nop + _wait_ge) replace coverage matmuls.
  - Tail: the TileContext drain/barrier is dropped entirely (one-shot NEFF,
    NRT drains the queues) and the Bass-init all-engine barrier is stripped
    (its orderings have >2us of slack here), pulling body start ~1us earlier.
"""

import numpy as np

B, R, D = 1024, 512, 256
N_CORES = 8
NB = 4                      # batch shards
NR = 2                      # rule shards
B2 = B // NB                # batch rows per core (256)
R2 = R // NR                # rules per core (256)
KT = D // 128               # contraction k-tiles
NRB = R2 // 128             # rule blocks per core (2)
BETA = 6.0
N_DUMMY_LONG = 10           # PE p-state warmup matmuls (free=256)
N_DUMMY_SHORT = 7           # PE p-state warmup matmuls (free=128)
NO_TAIL = True              # drop even the NOP-chain/drain/barrier (one-shot NEFF)
TRIM_TAIL = True            # skip Tile's sem-clear + second barrier (one-shot NEFF)
STRIP_INIT_BARRIER = False   # drop the Bass-init all-engine barrier (orderings it
                            # provides have >2us of slack in this graph)

_F32 = np.float32


def _single_wait_tile_context(nc, tile):
    """TileContext whose tail carries at most one sync wait per instruction."""
    from concourse.vector_clock import ScopedClock, VectorClock

    class SingleWaitTileContext(tile.TileContext):
        def _drain_and_barrier(self, tick_clock, wait_clock):
            if NO_TAIL:
                # one-shot NEFF: engine programs end at their last real
                # instruction; NRT tracks and drains pending DMA queues
                assert self.sems is not None
                popped = self.nc._tile_sem_poison_stack.pop()
                assert popped is self._sem_poison
                return
            gc = tick_clock.global_clock
            n = len(gc)
            for proc in range(n):
                if gc[proc] <= 0:
                    continue
                vec = VectorClock([gc[i] if i == proc else 0 for i in range(n)])
                inst = self.nc.sync.nop(nofuse=True)
                wait_clock.add_sem_waits(inst.ins, ScopedClock({None: vec}))
            # the NOP chain above already waited out every proc, so the drain
            # itself needs no waits (walrus would reject a multi-wait drain)
            self.nc.sync.drain()
            self.nc.all_engine_barrier()
            assert self.sems is not None
            popped = self.nc._tile_sem_poison_stack.pop()
            assert popped is self._sem_poison
            if not TRIM_TAIL:
                self.nc.clear_and_free_semaphores(
                    list(self.sems.allocated().values()))
                self.nc.all_engine_barrier()

    return SingleWaitTileContext(nc)


def _build_nc(c_uniform):
    import concourse.bass as bass
    import concourse.mybir as mybir
    from concourse import tile
    from concourse.tile_rust import add_dep_helper

    f32 = mybir.dt.float32
    bf16 = mybir.dt.bfloat16
    i16 = mybir.dt.int16
    AF = mybir.ActivationFunctionType

    nc = bass.Bass()
    if STRIP_INIT_BARRIER:
        # Bass.__init__ ends with const-AP memsets + all_engine_barrier.
        # The barrier's only job is ordering those memsets against later
        # cross-engine readers; our first const reader runs ~3us after the
        # memsets complete, so drop the barrier (drains + event semaphores).
        blk = nc.m.functions[0].blocks[0]
        drop = [i for i in blk.instructions
                if type(i).__name__ in ("InstDrain", "InstEventSemaphore")]
        for i in drop:
            blk.instructions.remove(i)
    # xlo: [khalf*(tau_lo - x) | c_k(bf16) | pad] per k-tile; c = khalf*
    # (tau_lo - tau_hi) is the tanh_hi bias, riding with x so tanh_hi
    # depends on nothing else
    d_xlo = nc.declare_dram_parameter("xlo", [128, KT * B2 + 2], bf16,
                                      isOutput=False)
    # ab0: [a_k0 | b_k0 | tb0(f32 as 2xbf16) | tb1 | w0 | w1 | scatter-idx | pad]
    d_ab0 = nc.declare_dram_parameter("ab0", [128, 2 * R2 + 8], bf16, isOutput=False)
    d_ab1 = nc.declare_dram_parameter("ab1", [128, 2 * R2], bf16, isOutput=False)
    d_y = nc.declare_dram_parameter("y", [1, B2], f32, isOutput=True)

    # Input DMAs fire PRE-TileContext, right after each engine's ring-config
    # moves (~1us before the tile body starts), with manual completion sems.
    # Destinations are raw SBUF tensors (like the const-AP pool); in-context
    # consumers order against them via one explicit post-scheduled wait each.
    # Scalar reaches its trigger first -> it carries xlo (the earliest-needed
    # tensor); the PE warmup dummies read an uninitialized raw tensor (their
    # results are never read, so the race with its memset is benign).
    t_xlo = nc.alloc_sbuf_tensor("xlo_sb", [128, KT * B2 + 2], bf16)
    t_ab0 = nc.alloc_sbuf_tensor("ab0_sb", [128, 2 * R2 + 8], bf16)
    t_ab1 = nc.alloc_sbuf_tensor("ab1_sb", [128, 2 * R2], bf16)
    t_dm = nc.alloc_sbuf_tensor("dummy_sb", [128, 512], bf16)
    t_yrow = nc.alloc_sbuf_tensor("yrow_sb", [128, 1, B2], f32)
    s_xlo = nc.alloc_semaphore("s_xlo")
    s_ab0 = nc.alloc_semaphore("s_ab0")
    s_ab1 = nc.alloc_semaphore("s_ab1")
    s_y = nc.alloc_semaphore("s_y")
    s_prep = nc.alloc_semaphore("s_prep")
    s_ydma = nc.alloc_semaphore("s_ydma")
    xlo, ab0, ab1, dummy = t_xlo.ap(), t_ab0.ap(), t_ab1.ap(), t_dm.ap()
    yrow = t_yrow.ap()
    nc.scalar.dma_start(xlo[:], d_xlo[:]).then_inc(s_xlo, 16)
    nc.sync.dma_start(ab0[:], d_ab0[:]).then_inc(s_ab0, 16)
    nc.gpsimd.dma_start(ab1[:], d_ab1[:]).then_inc(s_ab1, 16)
    nc.gpsimd.memset(dummy[:], 0.0)

    def chain(insts):
        for a, b_ in zip(insts, insts[1:]):
            add_dep_helper(b_.ins, a.ins, sync=False, reason="engine order")

    with _single_wait_tile_context(nc, tile) as tc:
        with (
            tc.tile_pool(name="sb", bufs=1) as sb,
            tc.tile_pool(name="ps", bufs=1, space="PSUM") as ps,
        ):
            warm = sb.tile([128, 1], f32, tag="warm")
            tlo = sb.tile([128, KT, B2], bf16, tag="tlo")
            thi = sb.tile([128, KT, B2], bf16, tag="thi")
            z = sb.tile([128, NRB, B2], bf16, tag="z")

            cst0 = nc.const_aps.aps[(f32, 0.0)]

            # ACT chain: warm (table load in DMA shadow), explicit waits on
            # the pre-context DMAs, tanh_lo, tanh_hi = tanh(-xlo + c)
            # (c = k/2*(tau_lo - tau_hi) rides in ab0 — no second x DMA)
            # waits on the pre-context DMA sems are attached POST-schedule
            # (the tile sim can't see the out-of-context increments and
            # would report deadlock)
            post_waits = []
            a_warm = nc.scalar.activation(warm[:], cst0, AF.Tanh)
            w_xlo = nc.scalar.nop(nofuse=True)
            post_waits.append((w_xlo, s_xlo))
            xin = xlo[:, 0:KT * B2].rearrange("p (k b) -> p k b", k=KT)
            a_tlo = nc.scalar.activation(tlo[:], xin, AF.Tanh)
            c_col = [xlo[:, KT * B2 + k:KT * B2 + k + 1] for k in range(KT)]
            if c_uniform:
                # c[d] identical for all d: one bias column serves both
                # k-tiles, so tanh_hi merges into a single activation
                a_thi = [nc.scalar.activation(thi[:], xin, AF.Tanh,
                                              bias=c_col[0], scale=-1.0)]
            else:
                a_thi = [nc.scalar.activation(thi[:, k, :], xin[:, k, :],
                                              AF.Tanh, bias=c_col[k],
                                              scale=-1.0)
                         for k in range(KT)]
            # ACT observes ab0 before the z sigmoids read their tb bias
            w_ab0 = nc.scalar.nop(nofuse=True)
            post_waits.append((w_ab0, s_ab0))

            # PE: pstate-warmup dummies (long ones early for ramp coverage,
            # short ones near the handoff for fine-grained queue drain),
            # then per-(k, side, rule-block) accumulation; explicit ab waits
            # replace the old coverage matmuls
            dps = ps.tile([128, 512], f32, tag="dps")
            ev = [ps.tile([128, B2], f32, name=f"ev{rb}", tag=f"ev{rb}")
                  for rb in range(NRB)]
            yps = ps.tile([1, B2], f32, tag="yps")

            pe = []
            for _ in range(N_DUMMY_LONG):
                pe.append(nc.tensor.matmul(dps[:, 0:256], dummy[:, 0:128],
                                           dummy[:, 0:256], start=True,
                                           stop=True))
            for _ in range(N_DUMMY_SHORT):
                pe.append(nc.tensor.matmul(dps[:, 0:128], dummy[:, 0:128],
                                           dummy[:, 0:128], start=True,
                                           stop=True))
            w_ab0p = nc.tensor.nop(nofuse=True)
            post_waits.append((w_ab0p, s_ab0))
            pe.append(w_ab0p)

            a_sl = [ab0[:, 0:R2], ab1[:, 0:R2]]          # A k-slices [d, r]
            b_sl = [ab0[:, R2:2 * R2], ab1[:, R2:2 * R2]]
            # lo side for both k, then wait ab1, then hi side; rb0's last
            # contribution precedes rb1's
            for k in range(KT):
                if k == 1:
                    w_ab1p = nc.tensor.nop(nofuse=True)
                    post_waits.append((w_ab1p, s_ab1))
                    pe.append(w_ab1p)
                for rb in range(NRB):
                    pe.append(nc.tensor.matmul(
                        ev[rb][:], a_sl[k][:, rb * 128:(rb + 1) * 128],
                        tlo[:, k, :], start=(k == 0), stop=False))
            # hi phase ordered so rb0's last contribution lands one matmul
            # earlier (z0 unblocks sooner; z1 stays gated by rb1's last)
            for rb in range(NRB):
                for k in range(KT):
                    pe.append(nc.tensor.matmul(
                        ev[rb][:], b_sl[k][:, rb * 128:(rb + 1) * 128],
                        thi[:, k, :], start=False, stop=(k == KT - 1)))

            # z = sigmoid(BETA*ev - BETA*t) with t as per-partition bias
            tb = [ab0[:, 2 * R2 + 2 * rb:2 * R2 + 2 * rb + 2].bitcast(f32)
                  for rb in range(NRB)]
            a_z = [nc.scalar.activation(z[:, rb, :], ev[rb][:], AF.Sigmoid,
                                        bias=tb[rb], scale=BETA)
                   for rb in range(NRB)]

            # head: y[1, b] += w_rb^T @ z_rb
            for rb in range(NRB):
                pe.append(nc.tensor.matmul(
                    yps[:], ab0[:, 2 * R2 + 4 + rb:2 * R2 + 5 + rb],
                    z[:, rb, :], start=(rb == 0), stop=(rb == NRB - 1)))
            chain(pe)

            # y path: DVE copies PSUM->SBUF, ACT triggers the output DMA
            # (it just finished z1 -- no cross-engine hop before the trigger)
            a_cp = nc.vector.tensor_scalar_mul(yrow[0:1, 0, :], yps[:], 1.0)
            i_y = nc.scalar.dma_start(d_y[:], yrow[0:1, 0, :])
            chain([a_warm, w_xlo, a_tlo] + a_thi + [w_ab0] + a_z + [i_y])

    for inst, sem in post_waits:
        inst._wait_ge(sem, 16)

    nc.finalize()
    return nc


def _fast_path_inputs(x, mask, e_low, e_high, tau_lo, tau_hi, kappa, t, head_w):
    """Per-core input maps.  Host work: parameter folding + transposes."""
    import ml_dtypes

    bf16 = ml_dtypes.bfloat16
    khalf = _F32(kappa) / _F32(2.0)

    def sig(v):
        return _F32(0.5) * (np.tanh(_F32(0.5) * v) + _F32(1.0))

    # folded params, feature-major: A/B [d, r]
    AT = (sig(mask) * np.tanh(e_low)).T.astype(_F32)      # (D, R)
    BT = (sig(mask) * np.tanh(e_high)).T.astype(_F32)
    xT = x.T.astype(_F32)                                  # (D, B)
    xloT = (khalf * (tau_lo[:, None] - xT)).astype(bf16)   # (D, B)
    c_d = (khalf * (tau_lo - tau_hi)).astype(_F32)         # (D,)
    w_row = head_w.reshape(R).astype(_F32)

    def dshape(a):  # (D, N) -> [128, KT, N] with d = k*128 + p
        return np.ascontiguousarray(
            a.reshape(KT, 128, a.shape[1]).transpose(1, 0, 2))

    c_cols = np.ascontiguousarray(c_d.reshape(KT, 128).T).astype(bf16)  # [128, KT]
    xlos = []
    for i in range(NB):
        xb = np.zeros((128, KT * B2 + 2), dtype=bf16)
        xb[:, 0:KT * B2] = dshape(
            xloT[:, i * B2:(i + 1) * B2]).reshape(128, KT * B2)
        xb[:, KT * B2:KT * B2 + 2] = c_cols
        xlos.append(xb)

    shards = []
    for j in range(NR):
        rs = slice(j * R2, (j + 1) * R2)
        a_k = dshape(AT[:, rs]).astype(bf16)               # [128, KT, R2]
        b_k = dshape(BT[:, rs]).astype(bf16)
        ab0 = np.empty((128, 2 * R2 + 8), dtype=bf16)
        idx16 = np.full(128, -1, dtype=np.int16)
        idx16[0] = 0
        ab0[:, 2 * R2 + 6] = idx16.view(bf16)
        ab0[:, 2 * R2 + 7] = 0
        ab0[:, 0:R2] = a_k[:, 0, :]
        ab0[:, R2:2 * R2] = b_k[:, 0, :]
        tb = np.ascontiguousarray(
            (-_F32(BETA) * t[rs]).astype(_F32).reshape(NRB, 128).T)  # [128, NRB]
        ab0[:, 2 * R2:2 * R2 + 4] = tb.view(bf16).reshape(128, NRB, 2).reshape(128, 4)
        ab0[:, 2 * R2 + 4:2 * R2 + 6] = np.ascontiguousarray(
            w_row[rs].reshape(NRB, 128).T).astype(bf16)
        ab1 = np.empty((128, 2 * R2), dtype=bf16)
        ab1[:, 0:R2] = a_k[:, 1, :]
        ab1[:, R2:2 * R2] = b_k[:, 1, :]
        shards.append({"ab0": ab0, "ab1": ab1})

    in_maps = []
    for c in range(N_CORES):
        i, j = c % NB, c // NB
        in_maps.append({"xlo": xlos[i], **shards[j]})
    return in_maps


def _reference_numpy(x, center, log_width, e_low, e_high, mask, log_kappa, t,
                     head_w, head_b):
    """General fallback, exact reference semantics in fp32 numpy (chunked)."""
    width = np.clip(np.exp(log_width, dtype=_F32), 1e-3, 50.0).astype(_F32)
    t_low = (center - _F32(0.5) * width).astype(_F32)
    t_high = (center + _F32(0.5) * width).astype(_F32)
    kappa = np.clip(np.exp(_F32(log_kappa)), 0.5, 50.0).astype(_F32)

    def sig(v):
        return _F32(0.5) * (np.tanh(_F32(0.5) * v) + _F32(1.0))

    m = sig(mask.astype(_F32))
    el = np.tanh(e_low.astype(_F32))
    eh = np.tanh(e_high.astype(_F32))
    out = np.empty(x.shape[0], dtype=_F32)
    for s in range(0, x.shape[0], 64):
        xc = x[s:s + 64].astype(_F32)
        low = sig(kappa * (t_low[None] - xc[:, None, :]))
        high = sig(kappa * (xc[:, None, :] - t_high[None]))
        evidence = np.sum(
            m[None] * (el[None] * (2 * low - 1) + eh[None] * (2 * high - 1)),
            axis=2, dtype=_F32)
        z = sig(_F32(BETA) * (evidence - t[None].astype(_F32)))
        out[s:s + 64] = z @ head_w.reshape(-1).astype(_F32) + _F32(head_b)
    return out


def kernel_with_stats(trace=False, **inputs):
    x = np.asarray(inputs["x"], dtype=_F32)
    center = np.asarray(inputs["center"], dtype=_F32)
    log_width = np.asarray(inputs["log_width"], dtype=_F32)
    e_low = np.asarray(inputs["e_low"], dtype=_F32)
    e_high = np.asarray(inputs["e_high"], dtype=_F32)
    mask = np.asarray(inputs["mask"], dtype=_F32)
    log_kappa = np.asarray(inputs["log_kappa"], dtype=_F32)
    t = np.asarray(inputs["t"], dtype=_F32)
    head_w = np.asarray(inputs["head_w"], dtype=_F32)
    head_b = np.asarray(inputs["head_b"], dtype=_F32)

    assert x.shape == (B, D) and mask.shape == (R, D)

    # fast-path structural check: thresholds constant across the rule axis
    width = np.clip(np.exp(log_width), 1e-3, 50.0).astype(_F32)
    t_low = (center - _F32(0.5) * width).astype(_F32)
    t_high = (center + _F32(0.5) * width).astype(_F32)
    if not (np.all(t_low == t_low[0:1]) and np.all(t_high == t_high[0:1])):
        out = _reference_numpy(x, center, log_width, e_low, e_high, mask,
                               log_kappa, t, head_w, head_b)
        return out, None

    from concourse.bass_utils import run_bass_kernel_spmd

    kappa = np.clip(np.exp(_F32(log_kappa)), 0.5, 50.0).astype(_F32)
    in_maps = _fast_path_inputs(
        x, mask, e_low, e_high, t_low[0], t_high[0], kappa, t, head_w)

    khalf = kappa / _F32(2.0)
    c_d = (khalf * (t_low[0] - t_high[0])).astype(_F32)
    nc = _build_nc(bool(np.all(c_d == c_d[0])))
    res = run_bass_kernel_spmd(nc, in_maps, list(range(N_CORES)), trace=trace)
    out = np.zeros(B, dtype=np.float64)
    for c in range(N_CORES):
        i = c % NB
        out[i * B2:(i + 1) * B2] += res.results[c]["y"].reshape(B2).astype(np.float64)
    out += float(head_b.reshape(-1)[0])
    return out.astype(_F32), res


def kernel(**inputs):
    out, _ = kernel_with_stats(**inputs)
    return out


# revision 54
# speedup vs baseline: 1.0797x; 1.0797x over previous
"""Trainium2 Bass kernel for nn_BiEvidenceNet.

Model (B=1024, R=512, D=256):
    width  = clip(exp(log_width), 1e-3, 50)                  (R,D)
    t_low  = center - width/2 ; t_high = center + width/2    (R,D)
    kappa  = clip(exp(log_kappa), 0.5, 50)                   scalar
    low    = sigmoid(kappa*(t_low - x))   high = sigmoid(kappa*(x - t_high))
    evidence[b,r] = sum_d m*(el*(2*low-1) + eh*(2*high-1))   m=sig(mask), el/eh=tanh(e_*)
    z = sigmoid(6*(evidence - t));  y = z @ head_w.T + head_b

Key identity: 2*sigmoid(u)-1 = tanh(u/2).  When t_low / t_high are constant
across the rule axis (true at init; verified at runtime), the (B,R,D)
broadcast collapses to two matmuls over the feature dim:
    evidence^T = A^T_{d,r} @ tanh(k/2*(tau_lo - x))^T + B^T @ tanh(k/2*(x - tau_hi))^T
with A = sig(mask)*tanh(e_low), B = sig(mask)*tanh(e_high) folded on the host
(they are pure parameter transforms, O(R*D)).

On-core layout is fully transposed vs. the obvious one: the contraction (d)
lives on partitions for the tanh tiles and A/B, and evidence lands in PSUM
with RULES on partitions and batch on the free axis.  That makes t a
per-partition bias column of the z sigmoid, and the head a single stationary
column (head_w) matmul -> y arrives as [1, B2] in PSUM, one DMA descriptor.

The second tanh operand is derived on-device: x_hi = -x_lo + c with
c[d] = kappa/2*(tau_lo - tau_hi), so only ONE x tensor is DMA'd; everything
moves as bf16 (tolerance is 2e-2; measured end-to-end rel err ~3e-3).

Sharding: 4 batch shards x 2 rule shards over the 8 cores; rule-sharded
partial y vectors are summed on the host (with head_b) during the gather.

Perf structure (walrus encodes at most ONE sync wait per instruction, so the
whole graph is arranged as single-semaphore chains; measured ~15.0us vs the
25.2us baseline):
  - Input DMAs fire PRE-TileContext with manual completion semaphores, so
    their triggers and DGE generation overlap the fixed ~6.5us NEFF startup;
    3 parallel queues (scalar HWDGE: xlo first - that engine reaches its
    trigger earliest; sync HWDGE: ab_k0+consts; gpsimd SWDGE: ab_k1), sharing
    ~130-200GB/s of fabric, so total bytes (~390KB/core) sets the phase.
  - ACT: warm tanh pulls the 1.3us table load into the DMA shadow; tanh_lo
    and (when c is uniform) tanh_hi are each ONE merged activation; z
    sigmoids take t and BETA via per-partition bias columns.
  - PE: bf16 everywhere; right-sized dummy matmuls during the DMA shadow
    keep the p-state ramped without delaying the real matmuls; explicit
    post-scheduled sem-waits (nop + _wait_ge) replace coverage matmuls.
  - Tail: the TileContext drain/barrier is dropped entirely (one-shot NEFF,
    NRT drains the queues) and the Bass-init all-engine barrier is stripped
    (its orderings have >2us of slack here), pulling body start ~1us earlier.
"""

import numpy as np

B, R, D = 1024, 512, 256
N_CORES = 8
NB = 4                      # batch shards
NR = 2                      # rule shards
B2 = B // NB                # batch rows per core (256)
R2 = R // NR                # rules per core (256)
KT = D // 128               # contraction k-tiles
NRB = R2 // 128             # rule blocks per core (2)
BETA = 6.0
N_DUMMY_LONG = 10           # PE p-state warmup matmuls (free=256)
N_DUMMY_SHORT = 5           # PE p-state warmup matmuls (free=128)
N_DUMMY_PRE = 4             # pre-TileContext warmup matmuls (free=256)
NO_TAIL = True              # drop even the NOP-chain/drain/barrier (one-shot NEFF)
TRIM_TAIL = True            # skip Tile's sem-clear + second barrier (one-shot NEFF)
STRIP_INIT_BARRIER = False   # drop the Bass-init all-engine barrier (orderings it
                            # provides have >2us of slack in this graph)

_F32 = np.float32


def _single_wait_tile_context(nc, tile):
    """TileContext whose tail carries at most one sync wait per instruction."""
    from concourse.vector_clock import ScopedClock, VectorClock

    class SingleWaitTileContext(tile.TileContext):
        def _drain_and_barrier(self, tick_clock, wait_clock):
            if NO_TAIL:
                # one-shot NEFF: engine programs end at their last real
                # instruction; NRT tracks and drains pending DMA queues
                assert self.sems is not None
                popped = self.nc._tile_sem_poison_stack.pop()
                assert popped is self._sem_poison
                return
            gc = tick_clock.global_clock
            n = len(gc)
            for proc in range(n):
                if gc[proc] <= 0:
                    continue
                vec = VectorClock([gc[i] if i == proc else 0 for i in range(n)])
                inst = self.nc.sync.nop(nofuse=True)
                wait_clock.add_sem_waits(inst.ins, ScopedClock({None: vec}))
            # the NOP chain above already waited out every proc, so the drain
            # itself needs no waits (walrus would reject a multi-wait drain)
            self.nc.sync.drain()
            self.nc.all_engine_barrier()
            assert self.sems is not None
            popped = self.nc._tile_sem_poison_stack.pop()
            assert popped is self._sem_poison
            if not TRIM_TAIL:
                self.nc.clear_and_free_semaphores(
                    list(self.sems.allocated().values()))
                self.nc.all_engine_barrier()

    return SingleWaitTileContext(nc)


def _build_nc(c_uniform):
    import concourse.bass as bass
    import concourse.mybir as mybir
    from concourse import tile
    from concourse.tile_rust import add_dep_helper

    f32 = mybir.dt.float32
    bf16 = mybir.dt.bfloat16
    AF = mybir.ActivationFunctionType

    nc = bass.Bass()
    if STRIP_INIT_BARRIER:
        # Bass.__init__ ends with const-AP memsets + all_engine_barrier.
        # The barrier's only job is ordering those memsets against later
        # cross-engine readers; our first const reader runs ~3us after the
        # memsets complete, so drop the barrier (drains + event semaphores).
        blk = nc.m.functions[0].blocks[0]
        drop = [i for i in blk.instructions
                if type(i).__name__ in ("InstDrain", "InstEventSemaphore")]
        for i in drop:
            blk.instructions.remove(i)
    # xlo: [khalf*(tau_lo - x) | c_k(bf16) | pad] per k-tile; c = khalf*
    # (tau_lo - tau_hi) is the tanh_hi bias, riding with x so tanh_hi
    # depends on nothing else
    d_xlo = nc.declare_dram_parameter("xlo", [128, KT * B2 + 2], bf16,
                                      isOutput=False)
    # ab0: [a_k0 | b_k0 | tb0(f32 as 2xbf16) | tb1 | w0 | w1 | scatter-idx | pad]
    d_ab0 = nc.declare_dram_parameter("ab0", [128, 2 * R2 + 8], bf16, isOutput=False)
    d_ab1 = nc.declare_dram_parameter("ab1", [128, 2 * R2], bf16, isOutput=False)
    d_y = nc.declare_dram_parameter("y", [1, B2], f32, isOutput=True)

    # Input DMAs fire PRE-TileContext, right after each engine's ring-config
    # moves (~1us before the tile body starts), with manual completion sems.
    # Destinations are raw SBUF tensors (like the const-AP pool); in-context
    # consumers order against them via one explicit post-scheduled wait each.
    # Scalar reaches its trigger first -> it carries xlo (the earliest-needed
    # tensor); the PE warmup dummies read an uninitialized raw tensor (their
    # results are never read, so the race with its memset is benign).
    t_xlo = nc.alloc_sbuf_tensor("xlo_sb", [128, KT * B2 + 2], bf16)
    t_ab0 = nc.alloc_sbuf_tensor("ab0_sb", [128, 2 * R2 + 8], bf16)
    t_ab1 = nc.alloc_sbuf_tensor("ab1_sb", [128, 2 * R2], bf16)
    t_dm = nc.alloc_sbuf_tensor("dummy_sb", [128, 512], bf16)
    t_yrow = nc.alloc_sbuf_tensor("yrow_sb", [1, B2], f32)
    s_xlo = nc.alloc_semaphore("s_xlo")
    s_ab0 = nc.alloc_semaphore("s_ab0")
    s_ab1 = nc.alloc_semaphore("s_ab1")
    xlo, ab0, ab1, dummy = t_xlo.ap(), t_ab0.ap(), t_ab1.ap(), t_dm.ap()
    yrow = t_yrow.ap()
    nc.scalar.dma_start(xlo[:], d_xlo[:]).then_inc(s_xlo, 16)
    nc.sync.dma_start(ab0[:], d_ab0[:]).then_inc(s_ab0, 16)
    nc.gpsimd.dma_start(ab1[:], d_ab1[:]).then_inc(s_ab1, 16)
    nc.gpsimd.memset(dummy[:], 0.0)
    # PE p-state warmup starts pre-context, right after PE's ring-config
    # moves (~0.6us before the tile body) — inputs are uninitialized SBUF,
    # results land in a scratch PSUM bank nobody reads
    dps_pre = nc.alloc_psum_tensor("dps_pre", [128, 256], mybir.dt.float32).ap()
    for _ in range(N_DUMMY_PRE):
        nc.tensor.matmul(dps_pre[:], dummy[:, 0:128], dummy[:, 0:256],
                         start=True, stop=True)

    def chain(insts):
        for a, b_ in zip(insts, insts[1:]):
            add_dep_helper(b_.ins, a.ins, sync=False, reason="engine order")

    with _single_wait_tile_context(nc, tile) as tc:
        with (
            tc.tile_pool(name="sb", bufs=1) as sb,
            tc.tile_pool(name="ps", bufs=1, space="PSUM") as ps,
        ):
            warm = sb.tile([128, 1], f32, tag="warm")
            tlo = sb.tile([128, KT, B2], bf16, tag="tlo")
            thi = sb.tile([128, KT, B2], bf16, tag="thi")
            z = sb.tile([128, NRB, B2], bf16, tag="z")

            cst0 = nc.const_aps.aps[(f32, 0.0)]

            # ACT chain: warm (table load in DMA shadow), explicit waits on
            # the pre-context DMAs, tanh_lo, tanh_hi = tanh(-xlo + c)
            # (c = k/2*(tau_lo - tau_hi) rides in ab0 — no second x DMA)
            # waits on the pre-context DMA sems are attached POST-schedule
            # (the tile sim can't see the out-of-context increments and
            # would report deadlock)
            post_waits = []
            a_warm = nc.scalar.activation(warm[:], cst0, AF.Tanh)
            w_xlo = nc.scalar.nop(nofuse=True)
            post_waits.append((w_xlo, s_xlo))
            xin = xlo[:, 0:KT * B2].rearrange("p (k b) -> p k b", k=KT)
            a_tlo = nc.scalar.activation(tlo[:], xin, AF.Tanh)
            c_col = [xlo[:, KT * B2 + k:KT * B2 + k + 1] for k in range(KT)]
            if c_uniform:
                # c[d] identical for all d: one bias column serves both
                # k-tiles, so tanh_hi merges into a single activation
                a_thi = [nc.scalar.activation(thi[:], xin, AF.Tanh,
                                              bias=c_col[0], scale=-1.0)]
            else:
                a_thi = [nc.scalar.activation(thi[:, k, :], xin[:, k, :],
                                              AF.Tanh, bias=c_col[k],
                                              scale=-1.0)
                         for k in range(KT)]
            # ACT observes ab0 before the z sigmoids read their tb bias
            w_ab0 = nc.scalar.nop(nofuse=True)
            post_waits.append((w_ab0, s_ab0))

            # PE: pstate-warmup dummies (long ones early for ramp coverage,
            # short ones near the handoff for fine-grained queue drain),
            # then per-(k, side, rule-block) accumulation; explicit ab waits
            # replace the old coverage matmuls
            dps = ps.tile([128, 512], f32, tag="dps")
            ev = [ps.tile([128, B2], f32, name=f"ev{rb}", tag=f"ev{rb}")
                  for rb in range(NRB)]
            yps = ps.tile([1, B2], f32, tag="yps")

            pe = []
            for _ in range(N_DUMMY_LONG):
                pe.append(nc.tensor.matmul(dps[:, 0:256], dummy[:, 0:128],
                                           dummy[:, 0:256], start=True,
                                           stop=True))
            for _ in range(N_DUMMY_SHORT):
                pe.append(nc.tensor.matmul(dps[:, 0:128], dummy[:, 0:128],
                                           dummy[:, 0:128], start=True,
                                           stop=True))
            w_ab0p = nc.tensor.nop(nofuse=True)
            post_waits.append((w_ab0p, s_ab0))
            pe.append(w_ab0p)

            a_sl = [ab0[:, 0:R2], ab1[:, 0:R2]]          # A k-slices [d, r]
            b_sl = [ab0[:, R2:2 * R2], ab1[:, R2:2 * R2]]
            # lo side for both k, then wait ab1, then hi side; rb0's last
            # contribution precedes rb1's
            for k in range(KT):
                if k == 1:
                    w_ab1p = nc.tensor.nop(nofuse=True)
                    post_waits.append((w_ab1p, s_ab1))
                    pe.append(w_ab1p)
                for rb in range(NRB):
                    pe.append(nc.tensor.matmul(
                        ev[rb][:], a_sl[k][:, rb * 128:(rb + 1) * 128],
                        tlo[:, k, :], start=(k == 0), stop=False))
            # hi phase ordered so rb0's last contribution lands one matmul
            # earlier (z0 unblocks sooner; z1 stays gated by rb1's last)
            for rb in range(NRB):
                for k in range(KT):
                    pe.append(nc.tensor.matmul(
                        ev[rb][:], b_sl[k][:, rb * 128:(rb + 1) * 128],
                        thi[:, k, :], start=False, stop=(k == KT - 1)))

            # z = sigmoid(BETA*ev - BETA*t) with t as per-partition bias
            tb = [ab0[:, 2 * R2 + 2 * rb:2 * R2 + 2 * rb + 2].bitcast(f32)
                  for rb in range(NRB)]
            a_z = [nc.scalar.activation(z[:, rb, :], ev[rb][:], AF.Sigmoid,
                                        bias=tb[rb], scale=BETA)
                   for rb in range(NRB)]

            # head: y[1, b] += w_rb^T @ z_rb
            for rb in range(NRB):
                pe.append(nc.tensor.matmul(
                    yps[:], ab0[:, 2 * R2 + 4 + rb:2 * R2 + 5 + rb],
                    z[:, rb, :], start=(rb == 0), stop=(rb == NRB - 1)))
            chain(pe)

            # y path: DVE copies PSUM->SBUF, ACT triggers the output DMA
            # (it just finished z1 -- no cross-engine hop before the trigger)
            a_cp = nc.vector.tensor_scalar_mul(yrow[:], yps[:], 1.0)
            i_y = nc.scalar.dma_start(d_y[:], yrow[:])
            chain([a_warm, w_xlo, a_tlo] + a_thi + [w_ab0] + a_z + [i_y])

    for inst, sem in post_waits:
        inst._wait_ge(sem, 16)

    nc.finalize()
    return nc


def _fast_path_inputs(x, mask, e_low, e_high, tau_lo, tau_hi, kappa, t, head_w):
    """Per-core input maps.  Host work: parameter folding + transposes."""
    import ml_dtypes

    bf16 = ml_dtypes.bfloat16
    khalf = _F32(kappa) / _F32(2.0)

    def sig(v):
        return _F32(0.5) * (np.tanh(_F32(0.5) * v) + _F32(1.0))

    # folded params, feature-major: A/B [d, r]
    AT = (sig(mask) * np.tanh(e_low)).T.astype(_F32)      # (D, R)
    BT = (sig(mask) * np.tanh(e_high)).T.astype(_F32)
    xT = x.T.astype(_F32)                                  # (D, B)
    xloT = (khalf * (tau_lo[:, None] - xT)).astype(bf16)   # (D, B)
    c_d = (khalf * (tau_lo - tau_hi)).astype(_F32)         # (D,)
    w_row = head_w.reshape(R).astype(_F32)

    def dshape(a):  # (D, N) -> [128, KT, N] with d = k*128 + p
        return np.ascontiguousarray(
            a.reshape(KT, 128, a.shape[1]).transpose(1, 0, 2))

    c_cols = np.ascontiguousarray(c_d.reshape(KT, 128).T).astype(bf16)  # [128, KT]
    xlos = []
    for i in range(NB):
        xb = np.zeros((128, KT * B2 + 2), dtype=bf16)
        xb[:, 0:KT * B2] = dshape(
            xloT[:, i * B2:(i + 1) * B2]).reshape(128, KT * B2)
        xb[:, KT * B2:KT * B2 + 2] = c_cols
        xlos.append(xb)

    shards = []
    for j in range(NR):
        rs = slice(j * R2, (j + 1) * R2)
        a_k = dshape(AT[:, rs]).astype(bf16)               # [128, KT, R2]
        b_k = dshape(BT[:, rs]).astype(bf16)
        ab0 = np.empty((128, 2 * R2 + 8), dtype=bf16)
        idx16 = np.full(128, -1, dtype=np.int16)
        idx16[0] = 0
        ab0[:, 2 * R2 + 6] = idx16.view(bf16)
        ab0[:, 2 * R2 + 7] = 0
        ab0[:, 0:R2] = a_k[:, 0, :]
        ab0[:, R2:2 * R2] = b_k[:, 0, :]
        tb = np.ascontiguousarray(
            (-_F32(BETA) * t[rs]).astype(_F32).reshape(NRB, 128).T)  # [128, NRB]
        ab0[:, 2 * R2:2 * R2 + 4] = tb.view(bf16).reshape(128, NRB, 2).reshape(128, 4)
        ab0[:, 2 * R2 + 4:2 * R2 + 6] = np.ascontiguousarray(
            w_row[rs].reshape(NRB, 128).T).astype(bf16)
        ab1 = np.empty((128, 2 * R2), dtype=bf16)
        ab1[:, 0:R2] = a_k[:, 1, :]
        ab1[:, R2:2 * R2] = b_k[:, 1, :]
        shards.append({"ab0": ab0, "ab1": ab1})

    in_maps = []
    for c in range(N_CORES):
        i, j = c % NB, c // NB
        in_maps.append({"xlo": xlos[i], **shards[j]})
    return in_maps


def _reference_numpy(x, center, log_width, e_low, e_high, mask, log_kappa, t,
                     head_w, head_b):
    """General fallback, exact reference semantics in fp32 numpy (chunked)."""
    width = np.clip(np.exp(log_width, dtype=_F32), 1e-3, 50.0).astype(_F32)
    t_low = (center - _F32(0.5) * width).astype(_F32)
    t_high = (center + _F32(0.5) * width).astype(_F32)
    kappa = np.clip(np.exp(_F32(log_kappa)), 0.5, 50.0).astype(_F32)

    def sig(v):
        return _F32(0.5) * (np.tanh(_F32(0.5) * v) + _F32(1.0))

    m = sig(mask.astype(_F32))
    el = np.tanh(e_low.astype(_F32))
    eh = np.tanh(e_high.astype(_F32))
    out = np.empty(x.shape[0], dtype=_F32)
    for s in range(0, x.shape[0], 64):
        xc = x[s:s + 64].astype(_F32)
        low = sig(kappa * (t_low[None] - xc[:, None, :]))
        high = sig(kappa * (xc[:, None, :] - t_high[None]))
        evidence = np.sum(
            m[None] * (el[None] * (2 * low - 1) + eh[None] * (2 * high - 1)),
            axis=2, dtype=_F32)
        z = sig(_F32(BETA) * (evidence - t[None].astype(_F32)))
        out[s:s + 64] = z @ head_w.reshape(-1).astype(_F32) + _F32(head_b)
    return out


def kernel_with_stats(trace=False, **inputs):
    x = np.asarray(inputs["x"], dtype=_F32)
    center = np.asarray(inputs["center"], dtype=_F32)
    log_width = np.asarray(inputs["log_width"], dtype=_F32)
    e_low = np.asarray(inputs["e_low"], dtype=_F32)
    e_high = np.asarray(inputs["e_high"], dtype=_F32)
    mask = np.asarray(inputs["mask"], dtype=_F32)
    log_kappa = np.asarray(inputs["log_kappa"], dtype=_F32)
    t = np.asarray(inputs["t"], dtype=_F32)
    head_w = np.asarray(inputs["head_w"], dtype=_F32)
    head_b = np.asarray(inputs["head_b"], dtype=_F32)

    assert x.shape == (B, D) and mask.shape == (R, D)

    # fast-path structural check: thresholds constant across the rule axis
    width = np.clip(np.exp(log_width), 1e-3, 50.0).astype(_F32)
    t_low = (center - _F32(0.5) * width).astype(_F32)
    t_high = (center + _F32(0.5) * width).astype(_F32)
    if not (np.all(t_low == t_low[0:1]) and np.all(t_high == t_high[0:1])):
        out = _reference_numpy(x, center, log_width, e_low, e_high, mask,
                               log_kappa, t, head_w, head_b)
        return out, None

    from concourse.bass_utils import run_bass_kernel_spmd

    kappa = np.clip(np.exp(_F32(log_kappa)), 0.5, 50.0).astype(_F32)
    in_maps = _fast_path_inputs(
        x, mask, e_low, e_high, t_low[0], t_high[0], kappa, t, head_w)

    khalf = kappa / _F32(2.0)
    c_d = (khalf * (t_low[0] - t_high[0])).astype(_F32)
    nc = _build_nc(bool(np.all(c_d == c_d[0])))
    res = run_bass_kernel_spmd(nc, in_maps, list(range(N_CORES)), trace=trace)
    out = np.zeros(B, dtype=np.float64)
    for c in range(N_CORES):
        i = c % NB
        out[i * B2:(i + 1) * B2] += res.results[c]["y"].reshape(B2).astype(np.float64)
    out += float(head_b.reshape(-1)[0])
    return out.astype(_F32), res


def kernel(**inputs):
    out, _ = kernel_with_stats(**inputs)
    return out
